# revision 4
# baseline (speedup 1.0000x reference)
"""Trainium2 Bass kernel for AdaptivePhysicallyConstrainedAttention.

Model (see problem reference): top-k-masked dense attention + residual + LayerNorm.
  mask  = top-3 columns of softmax(band_importance) -> additive -inf bias
  q,k,v = x @ W{q,k,v}.T + b        (B=4, L=2048, D=1024, H=16, hd=64)
  attn  = softmax(q k^T / 8 + bias) v ;  out = LN(x + attn @ Wo.T + bo) * gamma + beta

Sharding: 8 cores = (batch 4) x (query-halves 2). Each core computes K/V for its
full batch (duplicated within the pair) and attends its 1024 query rows — no
collectives. Host prep: top-k mask (tiny), weight transposes, bf16 casts, and a
per-core column permutation of x^T so every core's own query rows sit first
(keeps the graph SPMD-uniform).

On-device layout tricks:
  - scores computed transposed (S^T = K Q^T) so the column mask is a
    per-partition activation bias and exp output feeds the AV matmul as lhsT
  - V stored [k, head, 65] with a ones column -> AV matmul also produces the
    softmax denominator; normalization is a per-partition scale afterwards
  - matmuls in bf16 (fp32 accumulation), everything else fp32
"""

import sys

if "/opt/trn_rl_repo" not in sys.path:
    sys.path.insert(0, "/opt/trn_rl_repo")

import numpy as np
import ml_dtypes

import concourse.bass as bass  # noqa: F401  (registers engines)
import concourse.tile as tile
from concourse import bacc, mybir
from concourse.bass_utils import run_bass_kernel_spmd
from concourse.masks import make_identity

BF16 = mybir.dt.bfloat16
F32 = mybir.dt.float32
AF = mybir.ActivationFunctionType
OP = mybir.AluOpType

B, L, D, H, HD = 4, 2048, 1024, 16, 64
LQ = L // 2  # query rows per core
P = 128
NCORES = 8
TOPK = 3
SCALE = 1.0 / 8.0
MASK_BIAS = -10000.0
LN_EPS = 1e-5

NIT = D // P        # 8   contraction tiles over D
NOT = D // P        # 8   output tiles over D
NKT = L // P        # 16  key tiles
NQC = LQ // 512     # 2   query chunks of 512
NLT = LQ // P       # 8   own-row tiles


def build_nc():
    nc = bacc.Bacc(None, target_bir_lowering=False, debug=False)

    xT = nc.declare_dram_parameter("xT", [D, L], BF16, isOutput=False)
    xres = nc.declare_dram_parameter("xres", [LQ, D], F32, isOutput=False)
    wqT = nc.declare_dram_parameter("wqT", [D, D], BF16, isOutput=False)
    wkT = nc.declare_dram_parameter("wkT", [D, D], BF16, isOutput=False)
    wvT = nc.declare_dram_parameter("wvT", [D, D], BF16, isOutput=False)
    woT = nc.declare_dram_parameter("woT", [D, D], BF16, isOutput=False)
    bq = nc.declare_dram_parameter("bq", [P, NOT], F32, isOutput=False)
    bk = nc.declare_dram_parameter("bk", [P, NOT], F32, isOutput=False)
    bvb = nc.declare_dram_parameter("bvb", [P, D], F32, isOutput=False)
    biask = nc.declare_dram_parameter("biask", [P, NKT], F32, isOutput=False)
    gamb = nc.declare_dram_parameter("gamb", [P, D], F32, isOutput=False)
    betb = nc.declare_dram_parameter("betb", [P, D], F32, isOutput=False)
    out = nc.declare_dram_parameter("out", [LQ, D], F32, isOutput=True)

    with tile.TileContext(nc) as tc:
        with (
            tc.tile_pool(name="const", bufs=1) as constp,
            tc.tile_pool(name="big", bufs=1) as bigp,
            tc.tile_pool(name="wstream", bufs=2) as wsp,
            tc.tile_pool(name="ps", bufs=3, space="PSUM") as psp,
            tc.tile_pool(name="ctxps", bufs=1, space="PSUM") as ctxpsp,
            tc.tile_pool(name="trps", bufs=1, space="PSUM") as trpsp,
            tc.tile_pool(name="pt", bufs=4) as ptp,
            tc.tile_pool(name="small", bufs=8) as smallp,
            tc.tile_pool(name="io", bufs=2) as iop,
        ):
            # ---- constants ----
            biask_sb = constp.tile([P, NKT], F32, tag="biask")
            nc.sync.dma_start(out=biask_sb[:], in_=biask[:, :])
            bq_sb = constp.tile([P, NOT], F32, tag="bq")
            nc.sync.dma_start(out=bq_sb[:], in_=bq[:, :])
            bk_sb = constp.tile([P, NOT], F32, tag="bk")
            nc.sync.dma_start(out=bk_sb[:], in_=bk[:, :])
            bvb_sb = constp.tile([P, D], F32, tag="bvb")
            nc.sync.dma_start(out=bvb_sb[:], in_=bvb[:, :])
            gamb_sb = constp.tile([P, D], F32, tag="gamb")
            nc.sync.dma_start(out=gamb_sb[:], in_=gamb[:, :])
            betb_sb = constp.tile([P, D], F32, tag="betb")
            nc.sync.dma_start(out=betb_sb[:], in_=betb[:, :])
            eps_sb = constp.tile([P, 1], F32, tag="eps")
            nc.vector.memset(eps_sb[:], LN_EPS)
            ident = constp.tile([P, P], BF16, tag="ident")
            make_identity(nc, ident[:])

            # ---- big resident tensors ----
            xT_sb = bigp.tile([P, NIT, L], BF16, tag="xT")
            nc.sync.dma_start(
                out=xT_sb[:], in_=xT[:, :].rearrange("(t p) l -> p t l", p=P)
            )
            qT_sb = bigp.tile([P, NOT, LQ], BF16, tag="qT")
            kT_sb = bigp.tile([P, NOT, L], BF16, tag="kT")
            v_sb = bigp.tile([P, NKT, H, HD + 1], BF16, tag="v")
            ctxT_sb = bigp.tile([P, NIT, LQ], BF16, tag="ctxT")

            # ones column of the augmented V (softmax denominator trick)
            nc.vector.memset(v_sb[:, :, :, HD : HD + 1], 1.0)

            def proj_qk(ot, w_dram, bias_sb, dst_sb, n_lc):
                # dst^T[o-tile ot] = W x^T + b  for l in [0, 512*n_lc)
                wt = wsp.tile([P, NIT, P], BF16, tag="wqk")
                nc.sync.dma_start(
                    out=wt[:],
                    in_=w_dram[:, ot * P : (ot + 1) * P].rearrange(
                        "(t p) o -> p t o", p=P
                    ),
                )
                for lc in range(n_lc):
                    ps = psp.tile([P, 512], F32, tag="ps")
                    for it in range(NIT):
                        nc.tensor.matmul(
                            ps[:],
                            wt[:, it, :],
                            xT_sb[:, it, lc * 512 : (lc + 1) * 512],
                            start=(it == 0),
                            stop=(it == NIT - 1),
                        )
                    nc.vector.tensor_scalar(
                        out=dst_sb[:, ot, lc * 512 : (lc + 1) * 512],
                        in0=ps[:],
                        scalar1=bias_sb[:, ot : ot + 1],
                        scalar2=None,
                        op0=OP.add,
                    )

            def proj_v(og):
                # V natural rows, columns [512*og, 512*(og+1)) = heads 8og..8og+7
                wt = wsp.tile([P, NIT, 512], BF16, tag="wv")
                nc.sync.dma_start(
                    out=wt[:],
                    in_=wvT[:, og * 512 : (og + 1) * 512].rearrange(
                        "(t p) o -> p t o", p=P
                    ),
                )
                for lt in range(NKT):
                    ps = psp.tile([P, 512], F32, tag="ps")
                    for it in range(NIT):
                        nc.tensor.matmul(
                            ps[:],
                            xT_sb[:, it, lt * P : (lt + 1) * P],
                            wt[:, it, :],
                            start=(it == 0),
                            stop=(it == NIT - 1),
                        )
                    nc.vector.tensor_tensor(
                        out=v_sb[:, lt, 8 * og : 8 * og + 8, 0:HD],
                        in0=ps[:].rearrange("p (h d) -> p h d", h=8),
                        in1=bvb_sb[:, og * 512 : (og + 1) * 512].rearrange(
                            "p (h d) -> p h d", h=8
                        ),
                        op=OP.add,
                    )

            def attention_head(h):
                po = (h % 2) * HD
                ot = h // 2
                for qc in range(NQC):
                    ctx_ps = ctxpsp.tile([P, 4, 512], F32, tag="ctx")
                    # software-pipelined: S(kt) issued before AV(kt-1)
                    s_tiles = [None] * NKT
                    p_tiles = [None] * NKT

                    def s_step(kt):
                        ps = psp.tile([P, 512], F32, tag="ps")
                        nc.tensor.matmul(
                            ps[:],
                            kT_sb[po : po + HD, ot, kt * P : (kt + 1) * P],
                            qT_sb[po : po + HD, ot, qc * 512 : (qc + 1) * 512],
                            start=True,
                            stop=True,
                        )
                        pt = ptp.tile([P, 512], BF16, tag="pt")
                        nc.scalar.activation(
                            out=pt[:],
                            in_=ps[:],
                            func=AF.Exp,
                            bias=biask_sb[:, kt : kt + 1],
                            scale=SCALE,
                        )
                        s_tiles[kt] = ps
                        p_tiles[kt] = pt

                    def av_step(kt):
                        pt = p_tiles[kt]
                        for qs in range(4):
                            nc.tensor.matmul(
                                ctx_ps[:, qs, 0 : HD + 1],
                                pt[:, qs * P : (qs + 1) * P],
                                v_sb[:, kt, h, :],
                                start=(kt == 0),
                                stop=(kt == NKT - 1),
                            )

                    s_step(0)
                    for kt in range(1, NKT):
                        s_step(kt)
                        av_step(kt - 1)
                    av_step(NKT - 1)

                    # normalize by denominator, transpose into ctx^T
                    tr_ps = trpsp.tile([HD, 512], BF16, tag="tr")
                    for qs in range(4):
                        den = smallp.tile([P, 1], F32, tag="den")
                        nc.vector.reciprocal(den[:], ctx_ps[:, qs, HD : HD + 1])
                        cn = smallp.tile([P, HD], BF16, tag="cn")
                        nc.vector.tensor_scalar(
                            out=cn[:],
                            in0=ctx_ps[:, qs, 0:HD],
                            scalar1=den[:, 0:1],
                            scalar2=None,
                            op0=OP.mult,
                        )
                        nc.tensor.transpose(
                            tr_ps[:, qs * P : (qs + 1) * P], cn[:], ident[:]
                        )
                    nc.vector.tensor_copy(
                        out=ctxT_sb[po : po + HD, ot, qc * 512 : (qc + 1) * 512],
                        in_=tr_ps[:],
                    )

            # ---- pipelined emission: project an o-slice, then its two heads ----
            for ot in range(NOT):
                proj_qk(ot, wqT, bq_sb, qT_sb, NQC)
                proj_qk(ot, wkT, bk_sb, kT_sb, L // 512)
                if ot % 4 == 0:
                    proj_v(ot // 4)
                attention_head(2 * ot)
                attention_head(2 * ot + 1)

            # ---- output projection + residual + layernorm ----
            for oc in range(2):
                wt = wsp.tile([P, NIT, 512], BF16, tag="wo")
                nc.sync.dma_start(
                    out=wt[:],
                    in_=woT[:, oc * 512 : (oc + 1) * 512].rearrange(
                        "(t p) o -> p t o", p=P
                    ),
                )
                if oc == 0:
                    wo_tiles = [wt]
                else:
                    wo_tiles.append(wt)

            for lt in range(NLT):
                xr = iop.tile([P, D], F32, tag="xr")
                nc.sync.dma_start(out=xr[:], in_=xres[lt * P : (lt + 1) * P, :])
                y = iop.tile([P, D], F32, tag="y")
                for oc in range(2):
                    ps = psp.tile([P, 512], F32, tag="ps")
                    for it in range(NIT):
                        nc.tensor.matmul(
                            ps[:],
                            ctxT_sb[:, it, lt * P : (lt + 1) * P],
                            wo_tiles[oc][:, it, :],
                            start=(it == 0),
                            stop=(it == NIT - 1),
                        )
                    nc.vector.tensor_tensor(
                        out=y[:, oc * 512 : (oc + 1) * 512],
                        in0=ps[:],
                        in1=xr[:, oc * 512 : (oc + 1) * 512],
                        op=OP.add,
                    )
                stats = smallp.tile([P, 2, 6], F32, tag="stats")
                nc.vector.bn_stats(stats[:, 0, :], y[:, 0:512])
                nc.vector.bn_stats(stats[:, 1, :], y[:, 512:1024])
                mv = smallp.tile([P, 2], F32, tag="mv")
                nc.vector.bn_aggr(mv[:], stats[:])
                std = smallp.tile([P, 1], F32, tag="std")
                nc.scalar.activation(
                    out=std[:], in_=mv[:, 1:2], func=AF.Sqrt, bias=eps_sb[:, 0:1]
                )
                rstd = smallp.tile([P, 1], F32, tag="rstd")
                nc.vector.reciprocal(rstd[:], std[:])
                nc.vector.tensor_scalar(
                    out=y[:],
                    in0=y[:],
                    scalar1=mv[:, 0:1],
                    scalar2=rstd[:, 0:1],
                    op0=OP.subtract,
                    op1=OP.mult,
                )
                o_sb = iop.tile([P, D], F32, tag="o")
                nc.vector.tensor_tensor(out=o_sb[:], in0=y[:], in1=gamb_sb[:], op=OP.mult)
                nc.vector.tensor_tensor(out=o_sb[:], in0=o_sb[:], in1=betb_sb[:], op=OP.add)
                nc.sync.dma_start(out=out[lt * P : (lt + 1) * P, :], in_=o_sb[:])

    nc.compile()
    return nc


def host_prep(inputs):
    """Shard + lay out the full inputs into 8 per-core in_maps."""
    bf16 = ml_dtypes.bfloat16
    x = np.asarray(inputs["x"], dtype=np.float32)
    bi = np.asarray(inputs["band_importance"], dtype=np.float32)[0]
    idx = np.argpartition(-bi, TOPK)[:TOPK]  # top-k of softmax == top-k of logits
    bias_vec = np.zeros(L, np.float32)
    bias_vec[idx] = MASK_BIAS

    wqT = np.ascontiguousarray(np.asarray(inputs["Wq"], np.float32).T).astype(bf16)
    wkT = np.ascontiguousarray(np.asarray(inputs["Wk"], np.float32).T).astype(bf16)
    wvT = np.ascontiguousarray(np.asarray(inputs["Wv"], np.float32).T).astype(bf16)
    woT = np.ascontiguousarray(np.asarray(inputs["Wo"], np.float32).T).astype(bf16)
    bq = np.ascontiguousarray(np.asarray(inputs["bq"], np.float32).reshape(NOT, P).T)
    bk = np.ascontiguousarray(np.asarray(inputs["bk"], np.float32).reshape(NOT, P).T)
    bv = np.asarray(inputs["bv"], np.float32)
    bo = np.asarray(inputs["bo"], np.float32)
    gam = np.asarray(inputs["gamma"], np.float32)
    bet = np.asarray(inputs["beta"], np.float32)
    bvb = np.ascontiguousarray(np.broadcast_to(bv, (P, D)))
    gamb = np.ascontiguousarray(np.broadcast_to(gam, (P, D)))
    betb = np.ascontiguousarray(np.broadcast_to(bet, (P, D)))

    in_maps = []
    for c in range(NCORES):
        b, hh = c // 2, c % 2
        own = slice(hh * LQ, (hh + 1) * LQ)
        oth = slice((1 - hh) * LQ, (2 - hh) * LQ)
        xTb = x[b].T  # [D, L] view
        xT_c = np.concatenate([xTb[:, own], xTb[:, oth]], axis=1).astype(bf16)
        pb = np.concatenate([bias_vec[own], bias_vec[oth]])
        biask_c = np.ascontiguousarray(pb.reshape(NKT, P).T)
        xres_c = np.ascontiguousarray(x[b, own]) + bo[None, :]
        in_maps.append(
            {
                "xT": xT_c,
                "xres": xres_c,
                "wqT": wqT,
                "wkT": wkT,
                "wvT": wvT,
                "woT": woT,
                "bq": bq,
                "bk": bk,
                "bvb": bvb,
                "biask": biask_c,
                "gamb": gamb,
                "betb": betb,
            }
        )
    return in_maps


def assemble(results):
    out = np.empty((B, L, D), np.float32)
    for c in range(NCORES):
        b, hh = c // 2, c % 2
        out[b, hh * LQ : (hh + 1) * LQ, :] = results[c]["out"]
    return out


_NC_CACHE = None


def kernel(**inputs):
    global _NC_CACHE
    if _NC_CACHE is None:
        _NC_CACHE = build_nc()
    in_maps = host_prep(inputs)
    res = run_bass_kernel_spmd(_NC_CACHE, in_maps, core_ids=list(range(NCORES)))
    return assemble(res.results)
